# revision 1
# baseline (speedup 1.0000x reference)
"""LoRA generator kernel for Trainium2, sharded over 8 NeuronCores by layer.

Reference computation (see problem):
  pe = (condition @ W_proj + b_proj)                        (B=2, 224, 512)
  A  = (gelu(pe@WA1+bA1) @ WA2 + bA2) -> (B, L, 7, 16, 64)
  Bm = (gelu(pe@WB1+bB1) @ WB2 + bB2) -> (B, L, 7, 64, 16)
  out per (b, layer): concat over t of [tile_cols(A)*scA (16 x in_d),
                                        tile_rows(B)*scB (out_d x 16)]

Each core handles 4 layers (28 of the 224 projections). The big costs are
streaming its W_proj slice (22MB) in and writing its 36.8MB output slice; the
tiling/replication is done by DMA access patterns with step-0 (broadcast)
dims so the decoder outputs (56 rows x 4KB) fan out to ~37MB without compute.
The kernel is pipelined by layer: as soon as a layer's pe columns exist, that
layer is decoded, rearranged, and its output pieces stream out while later
layers' W_proj columns are still loading.

Piece-DMA layouts are chosen so each piece's descriptors stride partitions 8
apart (one per SDMA engine) and are 4KB each.
"""
import sys

sys.path.insert(0, "/opt/trn_rl_repo")

import numpy as np

import concourse.bass as bass
import concourse.bacc as bacc
import concourse.mybir as mybir
import concourse.tile as tile
from concourse.bass_utils import run_bass_kernel_spmd

F32 = mybir.dt.float32
F32R = mybir.dt.float32r

NCORES = 8
NUM_LAYERS = 32
RANK = 16
PED = 512
EMB = 384
T = 7
L = NUM_LAYERS // NCORES          # 4 layers per core
LT = L * T                        # 28 projections per core
ROWS = 2 * LT                     # 56 rows (b, l, t); row = (l*7+t)*2 + b
WP_COLS = LT * PED                # 14336
RPL = 2 * T                       # 14 rows per layer

IN_DS = [4096, 4096, 4096, 4096, 4096, 4096, 11008]
OUT_DS = [4096, 1024, 1024, 4096, 11008, 11008, 4096]
A_SIZES = [16 * d for d in IN_DS]
B_SIZES = [16 * d for d in OUT_DS]
LAYER_SIZE = sum(A_SIZES) + sum(B_SIZES)   # 1150976
OFF_A = []
OFF_B = []
_o = 0
for _t in range(T):
    OFF_A.append(_o)
    _o += A_SIZES[_t]
    OFF_B.append(_o)
    _o += B_SIZES[_t]
OUT_SZ = 2 * L * LAYER_SIZE

N_ROUNDS = 7                       # W_proj column rounds (4 lt-blocks each)
LT_PER_RD = LT // N_ROUNDS         # 4
RCOLS = LT_PER_RD * PED            # 2048
# after round rd, these layers are fully covered (7 lt each)
LAYER_DONE = {1: [0], 3: [1], 5: [2], 6: [3]}


PB_L = [0, 32, 64, 64]           # partition base per layer (engine ops need 0/32/64)
ACOL = [0, 0, 0, 1024]           # oa column offset per layer
BCOL = [0, 0, 0, 1024]           # ob / scaled-bias column offset per layer


def _prow(row):
    """Physical partition of a row (layer 3 shares 64.. with layer 2)."""
    return PB_L[row // RPL] + (row % RPL)


def _gbase(g):
    """First partition of row-group g (7 rows each, g = row // 7)."""
    return PB_L[g // 2] + 7 * (g % 2)


def _build_nc():
    nc = bacc.Bacc(None, target_bir_lowering=False, debug=False)

    cond = nc.declare_dram_parameter("cond", [128, 6], F32, isOutput=False)
    wp = nc.declare_dram_parameter("wp", [EMB, WP_COLS], F32, isOutput=False)
    bpt = nc.declare_dram_parameter("bpt", [128, 4 * LT], F32, isOutput=False)
    wa1 = nc.declare_dram_parameter("wa1", [128, 1024], F32, isOutput=False)
    wb1 = nc.declare_dram_parameter("wb1", [128, 1024], F32, isOutput=False)
    wa2 = nc.declare_dram_parameter("wa2", [128, 2048], F32, isOutput=False)
    wb2 = nc.declare_dram_parameter("wb2", [128, 2048], F32, isOutput=False)
    ba1 = nc.declare_dram_parameter("ba1", [128, 2], F32, isOutput=False)
    bb1 = nc.declare_dram_parameter("bb1", [128, 2], F32, isOutput=False)
    sca = nc.declare_dram_parameter("sca", [128, ROWS], F32, isOutput=False)
    scb = nc.declare_dram_parameter("scb", [128, ROWS], F32, isOutput=False)
    sba2 = nc.declare_dram_parameter("sba2", [128, 2048], F32, isOutput=False)
    sbb2 = nc.declare_dram_parameter("sbb2", [128, 2048], F32, isOutput=False)
    ident = nc.declare_dram_parameter("ident", [128, 2], F32, isOutput=False)
    out = nc.declare_dram_parameter("out", [OUT_SZ], F32, isOutput=True)

    with tile.TileContext(nc) as tc:
        with (
            tc.tile_pool(name="const", bufs=1) as cpool,
            tc.tile_pool(name="wp", bufs=2) as wpool,
            tc.tile_pool(name="work", bufs=1) as wkpool,
            tc.tile_pool(name="pe2", bufs=2) as pe2pool,
            tc.tile_pool(name="ps", bufs=1, space="PSUM") as ps,
        ):
            cond_sb = cpool.tile([128, 6], F32R)
            nc.gpsimd.dma_start(cond_sb[:], cond[:])
            bpt_sb = cpool.tile([128, 4 * LT], F32)
            nc.sync.dma_start(bpt_sb[:], bpt[:])
            wa1_sb = cpool.tile([128, 1024], F32)
            nc.sync.dma_start(wa1_sb[:], wa1[:])
            wb1_sb = cpool.tile([128, 1024], F32)
            nc.sync.dma_start(wb1_sb[:], wb1[:])
            wa2_sb = cpool.tile([128, 2048], F32)
            nc.sync.dma_start(wa2_sb[:], wa2[:])
            wb2_sb = cpool.tile([128, 2048], F32)
            nc.sync.dma_start(wb2_sb[:], wb2[:])
            ba1_sb = cpool.tile([128, 2], F32)
            nc.sync.dma_start(ba1_sb[:], ba1[:])
            bb1_sb = cpool.tile([128, 2], F32)
            nc.sync.dma_start(bb1_sb[:], bb1[:])
            sca_sb = cpool.tile([128, ROWS], F32)
            nc.sync.dma_start(sca_sb[:], sca[:])
            scb_sb = cpool.tile([128, ROWS], F32)
            nc.sync.dma_start(scb_sb[:], scb[:])
            sba2_sb = cpool.tile([128, 2048], F32)
            nc.sync.dma_start(sba2_sb[:], sba2[:])
            sbb2_sb = cpool.tile([128, 2048], F32)
            nc.sync.dma_start(sbb2_sb[:], sbb2[:])
            ident_sb = cpool.tile([128, 2], F32)
            nc.sync.dma_start(ident_sb[:], ident[:])

            # long-lived work tiles
            pe_sb = [
                wkpool.tile([128, ROWS], F32, tag=f"pe_sb{mc}", name=f"pe_sb{mc}")
                for mc in range(4)
            ]
            oa = wkpool.tile([128, 2048], F32)     # decoder A out
            ob_sb = wkpool.tile([128, 2048], F32)  # decoder B out
            aexp = wkpool.tile([128, 8 * 1024], F32)
            bexp = wkpool.tile([128, 8 * 1024], F32)
            pa = oa[:, :].ap[0][0]
            pax = aexp[:, :].ap[0][0]
            pob = ob_sb[:, :].ap[0][0]
            pbx = bexp[:, :].ap[0][0]
            oa_t = oa[:, :].tensor
            aexp_t = aexp[:, :].tensor
            ob_t = ob_sb[:, :].tensor
            bexp_t = bexp[:, :].tensor

            def decode_layer(l):
                """Decoder MLPs + rearrange + piece DMAs for layer l."""
                c0 = RPL * l              # first row / pe_sb column of the layer
                pb = PB_L[l]              # partition base (0/32/64)
                acol, bcol = ACOL[l], BCOL[l]
                for dec, (w1_sb, b1_sb, w2_sb, sc_sb, sb2_sb) in enumerate(
                    [
                        (wa1_sb, ba1_sb, wa2_sb, sca_sb, sba2_sb),
                        (wb1_sb, bb1_sb, wb2_sb, scb_sb, sbb2_sb),
                    ]
                ):
                    h_sb = []
                    for mc in range(2):
                        hp = ps.tile([128, RPL], F32, tag=f"h{mc}", name=f"hp{mc}")
                        for kc in range(4):
                            nc.tensor.matmul(
                                hp[:],
                                w1_sb[:, kc * 256 + mc * 128 : kc * 256 + (mc + 1) * 128],
                                pe_sb[kc][:, c0 : c0 + RPL],
                                start=(kc == 0),
                                stop=(kc == 3),
                            )
                        hs = wkpool.tile(
                            [128, RPL], F32, tag=f"h_sb{dec}{mc}", name=f"hs{dec}{mc}"
                        )
                        nc.scalar.activation(
                            hs[:], hp[:], mybir.ActivationFunctionType.Gelu,
                            bias=b1_sb[:, mc : mc + 1],
                        )
                        nc.vector.tensor_mul(hs[:], hs[:], sc_sb[:, c0 : c0 + RPL])
                        h_sb.append(hs)
                    for nh in range(2):
                        op = ps.tile([128, 512], F32, tag=f"o{nh}", name=f"op{nh}")
                        for kc in range(2):
                            nc.tensor.matmul(
                                op[pb : pb + RPL, :],
                                h_sb[kc][:],
                                w2_sb[:, kc * 1024 + nh * 512 : kc * 1024 + (nh + 1) * 512],
                                start=(kc == 0),
                                stop=(kc == 1),
                            )
                        tgt = oa if dec == 0 else ob_sb
                        coff = acol if dec == 0 else bcol
                        nc.vector.tensor_add(
                            tgt[pb : pb + RPL, coff + nh * 512 : coff + (nh + 1) * 512],
                            op[pb : pb + RPL, :],
                            sb2_sb[pb : pb + RPL, bcol + nh * 512 : bcol + (nh + 1) * 512],
                        )

                # rearrange into engine-striding exp layouts:
                # aexp[r*8 + slot, g*1024 + du] = oa[prow(7g+slot), r*256 + du%256]
                # bexp[k*8 + slot, g*1024 + j]  = ob[prow(7g+slot), j]  (all k)
                for g in (2 * l, 2 * l + 1):
                    gb = _gbase(g)
                    for r in range(16):
                        dst = bass.AP(
                            aexp_t, r * 8 * pax + g * 1024, [[pax, 7], [1, 256]]
                        )
                        src = bass.AP(
                            oa_t, gb * pa + acol + r * 64, [[pa, 7], [0, 4], [1, 64]]
                        )
                        nc.sync.dma_start(dst, src)
                    for w in (256, 512):
                        dst = bass.AP(aexp_t, g * 1024 + w, [[pax, 128], [1, w]])
                        src = bass.AP(aexp_t, g * 1024, [[pax, 128], [1, w]])
                        nc.sync.dma_start(dst, src)
                    dstb = bass.AP(bexp_t, g * 1024, [[pbx, 7], [1, 1024]])
                    srcb = bass.AP(ob_t, gb * pob + bcol, [[pob, 7], [1, 1024]])
                    nc.scalar.dma_start(dstb, srcb)
                for np_ in (8, 16, 32, 64):
                    dst = bass.AP(
                        bexp_t, np_ * pbx + 2 * l * 1024, [[pbx, np_], [1, 2048]]
                    )
                    src = bass.AP(bexp_t, 2 * l * 1024, [[pbx, np_], [1, 2048]])
                    nc.scalar.dma_start(dst, src)

                # piece DMAs (4KB descriptors striding all 16 engines)
                for row in range(RPL * l, RPL * (l + 1)):
                    lt, b = row // 2, row % 2
                    t = lt % T
                    g, slot = row // 7, row % 7
                    in_d, out_d = IN_DS[t], OUT_DS[t]
                    base = (b * L + l) * LAYER_SIZE + OFF_A[t]
                    nf, tail = in_d // 1024, in_d % 1024
                    dst = bass.AP(out, base, [[in_d, 16], [1024, nf], [1, 1024]])
                    src = bass.AP(
                        aexp_t,
                        slot * pax + g * 1024,
                        [[8 * pax, 16], [0, nf], [1, 1024]],
                    )
                    nc.sync.dma_start(dst, src)
                    if tail:
                        dst = bass.AP(out, base + nf * 1024, [[in_d, 16], [1, tail]])
                        src = bass.AP(
                            aexp_t, slot * pax + g * 1024, [[8 * pax, 16], [1, tail]]
                        )
                        nc.sync.dma_start(dst, src)
                    base = (b * L + l) * LAYER_SIZE + OFF_B[t]
                    nb = out_d // 64
                    nbf, nbt = nb // 16, nb % 16
                    dst = bass.AP(
                        out, base, [[1024, 16], [16 * 1024, nbf], [1, 1024]]
                    )
                    src = bass.AP(
                        bexp_t,
                        slot * pbx + g * 1024,
                        [[8 * pbx, 16], [0, nbf], [1, 1024]],
                    )
                    nc.scalar.dma_start(dst, src)
                    if nbt:
                        dst = bass.AP(
                            out, base + nbf * 16 * 1024, [[1024, nbt], [1, 1024]]
                        )
                        src = bass.AP(
                            bexp_t, slot * pbx + g * 1024, [[8 * pbx, nbt], [1, 1024]]
                        )
                        nc.scalar.dma_start(dst, src)

            # ---- main pipeline: stream W_proj, compute pe, decode per layer ----
            for rd in range(N_ROUNDS):
                wp_t = []
                for kc in range(3):
                    t_ = wpool.tile([128, RCOLS], F32R, tag=f"wp{kc}", name=f"wp_t{kc}")
                    nc.gpsimd.dma_start(
                        t_[:],
                        wp[kc * 128 : (kc + 1) * 128, rd * RCOLS : (rd + 1) * RCOLS],
                    )
                    wp_t.append(t_)
                pe2_sb = pe2pool.tile([2, RCOLS], F32, tag="pe2sb", name="pe2_sb")
                for ltl in range(LT_PER_RD):
                    p2 = ps.tile([2, PED], F32, tag=f"p2{ltl % 2}", name="pe2_ps")
                    for kc in range(3):
                        nc.tensor.matmul(
                            p2[:],
                            cond_sb[:, kc * 2 : kc * 2 + 2],
                            wp_t[kc][:, ltl * PED : (ltl + 1) * PED],
                            start=(kc == 0),
                            stop=(kc == 2),
                        )
                    nc.vector.tensor_copy(pe2_sb[:, ltl * PED : (ltl + 1) * PED], p2[:])
                for ltl in range(LT_PER_RD):
                    lt = rd * LT_PER_RD + ltl
                    for mc in range(4):
                        tr = ps.tile([128, 2], F32, tag=f"tr{mc % 2}", name="tr_ps")
                        nc.tensor.transpose(
                            tr[:],
                            pe2_sb[:, ltl * PED + mc * 128 : ltl * PED + (mc + 1) * 128],
                            ident_sb[0:2, 0:2],
                        )
                        # pe_T with b_proj bias (per-partition, same for both b)
                        nc.vector.tensor_scalar_add(
                            pe_sb[mc][:, 2 * lt : 2 * lt + 2],
                            tr[:],
                            bpt_sb[:, mc * LT + lt : mc * LT + lt + 1],
                        )
                for l in LAYER_DONE.get(rd, []):
                    decode_layer(l)

    nc.finalize()
    return nc


_NC = None


def _get_nc():
    global _NC
    if _NC is None:
        _NC = _build_nc()
    return _NC


def _marshal(inputs):
    """Build the per-core input maps from full inputs."""
    condition = np.asarray(inputs["condition"], np.float32)
    W_proj = np.asarray(inputs["W_proj"], np.float32)
    b_proj = np.asarray(inputs["b_proj"], np.float32)
    WA1 = np.asarray(inputs["WA1"], np.float32)
    bA1 = np.asarray(inputs["bA1"], np.float32)
    WA2 = np.asarray(inputs["WA2"], np.float32)
    bA2 = np.asarray(inputs["bA2"], np.float32)
    WB1 = np.asarray(inputs["WB1"], np.float32)
    bB1 = np.asarray(inputs["bB1"], np.float32)
    WB2 = np.asarray(inputs["WB2"], np.float32)
    bB2 = np.asarray(inputs["bB2"], np.float32)
    scales = np.asarray(inputs["scales"], np.float32)

    cond_arr = np.zeros((128, 6), np.float32)
    for kc in range(3):
        cond_arr[:, kc * 2 : kc * 2 + 2] = condition[:, kc * 128 : (kc + 1) * 128].T
    wa1_arr = np.zeros((128, 1024), np.float32)
    wb1_arr = np.zeros((128, 1024), np.float32)
    for kc in range(4):
        wa1_arr[:, kc * 256 : (kc + 1) * 256] = WA1[kc * 128 : (kc + 1) * 128, :]
        wb1_arr[:, kc * 256 : (kc + 1) * 256] = WB1[kc * 128 : (kc + 1) * 128, :]
    wa2_arr = np.zeros((128, 2048), np.float32)
    wb2_arr = np.zeros((128, 2048), np.float32)
    for kc in range(2):
        wa2_arr[:, kc * 1024 : (kc + 1) * 1024] = WA2[kc * 128 : (kc + 1) * 128, :]
        wb2_arr[:, kc * 1024 : (kc + 1) * 1024] = WB2[kc * 128 : (kc + 1) * 128, :]
    ba1_arr = np.ascontiguousarray(bA1.reshape(2, 128).T)
    bb1_arr = np.ascontiguousarray(bB1.reshape(2, 128).T)
    ident_arr = np.zeros((128, 2), np.float32)
    ident_arr[0, 0] = 1.0
    ident_arr[1, 1] = 1.0

    in_maps = []
    for c in range(NCORES):
        lt0 = c * LT
        wp_c = np.ascontiguousarray(W_proj[:, lt0 * PED : (lt0 + LT) * PED])
        bp_c = b_proj[lt0 * PED : (lt0 + LT) * PED].reshape(LT, 4, 128)
        bpt_arr = np.zeros((128, 4 * LT), np.float32)
        for lt in range(LT):
            for mc in range(4):
                bpt_arr[:, mc * LT + lt] = bp_c[lt, mc, :]
        sca_row = np.zeros(ROWS, np.float32)
        scb_row = np.zeros(ROWS, np.float32)
        for row in range(ROWS):
            lt = row // 2
            sca_row[row] = scales[lt0 + lt, 0]
            scb_row[row] = scales[lt0 + lt, 1]
        sca_arr = np.broadcast_to(sca_row[None, :], (128, ROWS)).copy()
        scb_arr = np.broadcast_to(scb_row[None, :], (128, ROWS)).copy()
        sba2_arr = np.zeros((128, 2048), np.float32)
        sbb2_arr = np.zeros((128, 2048), np.float32)
        for row in range(ROWS):
            p = _prow(row)
            blk = BCOL[row // RPL]
            sba2_arr[p, blk : blk + 1024] = sca_row[row] * bA2
            sbb2_arr[p, blk : blk + 1024] = scb_row[row] * bB2
        in_maps.append(
            {
                "cond": cond_arr,
                "wp": wp_c,
                "bpt": bpt_arr,
                "wa1": wa1_arr,
                "wb1": wb1_arr,
                "wa2": wa2_arr,
                "wb2": wb2_arr,
                "ba1": ba1_arr,
                "bb1": bb1_arr,
                "sca": sca_arr,
                "scb": scb_arr,
                "sba2": sba2_arr,
                "sbb2": sbb2_arr,
                "ident": ident_arr,
            }
        )
    return in_maps


def _ensure_ntff_hook():
    """Register the axon NTFF profile hook if the boot didn't (module was
    missing at boot time)."""
    import types

    ah = sys.modules.get("antenv.axon_hooks")
    if ah is None:
        ah = types.ModuleType("antenv.axon_hooks")
        ah._hook = None

        def _set(h, _m=ah):
            _m._hook = h

        def _get(_m=ah):
            return _m._hook

        ah.set_axon_ntff_profile_hook = _set
        ah.get_axon_ntff_profile_hook = _get
        sys.modules["antenv.axon_hooks"] = ah
        import antenv

        antenv.axon_hooks = ah
    if ah.get_axon_ntff_profile_hook() is None:
        if "/root/.axon_site" not in sys.path:
            sys.path.insert(0, "/root/.axon_site")
        from trn_agent_boot.trn_boot import _ntff_profile_via_ctypes

        hook = _ntff_profile_via_ctypes("/opt/axon/libaxon_pjrt.so")
        if hook is not None:
            ah.set_axon_ntff_profile_hook(hook)


def _run(inputs, trace=False):
    if trace:
        _ensure_ntff_hook()
    nc = _get_nc()
    in_maps = _marshal(inputs)
    res = run_bass_kernel_spmd(nc, in_maps, list(range(NCORES)), trace=trace)
    full = np.empty((2, NUM_LAYERS, LAYER_SIZE), np.float32)
    for c in range(NCORES):
        full[:, c * L : (c + 1) * L, :] = res.results[c]["out"].reshape(
            2, L, LAYER_SIZE
        )
    return full.reshape(2, -1), res


def kernel(**inputs) -> np.ndarray:
    out, _ = _run(inputs, trace=False)
    return out



# revision 25
# speedup vs baseline: 1.1774x; 1.1774x over previous
"""LoRA generator kernel for Trainium2, sharded over 8 NeuronCores by layer.

Reference computation:
  pe = (condition @ W_proj + b_proj)                        (B=2, 224, 512)
  A  = (gelu(pe@WA1+bA1) @ WA2 + bA2) -> (B, L, 7, 16, 64)
  Bm = (gelu(pe@WB1+bB1) @ WB2 + bB2) -> (B, L, 7, 64, 16)
  out per (b, layer): concat over t of [tile_cols(A)*scA (16 x in_d),
                                        tile_rows(B)*scB (out_d x 16)]

Each core handles 4 layers (28 of the 224 projections). All HBM traffic is
bf16 (W_proj cast on host, output upcast on host), halving the dominant DMA
bytes. Output pieces are written with 16KB descriptors: the decoder outputs
are scattered (one small SBUF->SBUF DMA per piece family) into wide tiles
whose partition rows hold the piece content base, then the periodic tiling
is materialized in-place by log2 doubling copies on the Vector/Scalar
engines, and each piece row streams out as contiguous 16KB reads.
"""
import sys

sys.path.insert(0, "/opt/trn_rl_repo")

import numpy as np
import ml_dtypes

import concourse.bass as bass
import concourse.bacc as bacc
import concourse.mybir as mybir
import concourse.tile as tile
from concourse.bass_utils import run_bass_kernel_spmd

F32 = mybir.dt.float32
BF16 = mybir.dt.bfloat16
NPBF16 = ml_dtypes.bfloat16

NCORES = 8
NUM_LAYERS = 32
RANK = 16
PED = 512
EMB = 384
T = 7
L = NUM_LAYERS // NCORES          # 4 layers per core
LT = L * T                        # 28 projections per core
RPL = 2 * T                       # 14 rows per layer; row = t*2 + b
CHUNK = T * PED                   # 3584 W_proj cols per layer

IN_DS = [4096, 4096, 4096, 4096, 4096, 4096, 11008]
OUT_DS = [4096, 1024, 1024, 4096, 11008, 11008, 4096]
A_SIZES = [16 * d for d in IN_DS]
B_SIZES = [16 * d for d in OUT_DS]
LAYER_SIZE = sum(A_SIZES) + sum(B_SIZES)   # 1150976
OFF_A = []
OFF_B = []
_o = 0
for _t in range(T):
    OFF_A.append(_o)
    _o += A_SIZES[_t]
    OFF_B.append(_o)
    _o += B_SIZES[_t]
OUT_SZ = 2 * L * LAYER_SIZE
LLS = L * LAYER_SIZE               # out stride between b=0 and b=1
WV = 8192                          # A/B tile row width (16KB bf16)

# CoreSim has no Gelu; debug harnesses set this to validate dataflow
SIM_GELU_IDENTITY = False

# Per-layer projections are processed in "stage order" s = rt(t) so that the
# t=6 (down_proj) rows land on partitions 0 and 7: its rank-scatter AP then
# has a stride-7 partition dim with zero offset, which the dependency
# tracker handles (k*pitch stride + nonzero partition offset does not).
RT = [1, 2, 3, 4, 5, 6, 0]         # stage slot of projection t
TOF = [6, 0, 1, 2, 3, 4, 5]        # projection t in stage slot s


def _build_nc():
    nc = bacc.Bacc(None, target_bir_lowering=False, debug=False)

    cond = nc.declare_dram_parameter("cond", [128, 6], BF16, isOutput=False)
    wp = nc.declare_dram_parameter("wp", [EMB, LT * PED], BF16, isOutput=False)
    bias2 = nc.declare_dram_parameter("bias2", [2, L * CHUNK], BF16, isOutput=False)
    wa1 = nc.declare_dram_parameter("wa1", [128, 1024], BF16, isOutput=False)
    wb1 = nc.declare_dram_parameter("wb1", [128, 1024], BF16, isOutput=False)
    wa2 = nc.declare_dram_parameter("wa2", [128, 2048], BF16, isOutput=False)
    wb2 = nc.declare_dram_parameter("wb2", [128, 2048], BF16, isOutput=False)
    ba1 = nc.declare_dram_parameter("ba1", [128, 2], F32, isOutput=False)
    bb1 = nc.declare_dram_parameter("bb1", [128, 2], F32, isOutput=False)
    sca = nc.declare_dram_parameter("sca", [128, 2 * LT], BF16, isOutput=False)
    scb = nc.declare_dram_parameter("scb", [128, 2 * LT], BF16, isOutput=False)
    sba2 = nc.declare_dram_parameter("sba2", [16, L * 1024], BF16, isOutput=False)
    sbb2 = nc.declare_dram_parameter("sbb2", [16, L * 1024], BF16, isOutput=False)
    ident = nc.declare_dram_parameter("ident", [16, 16], BF16, isOutput=False)
    out = nc.declare_dram_parameter("out", [OUT_SZ], BF16, isOutput=True)

    with tile.TileContext(nc) as tc:
        with (
            tc.tile_pool(name="const", bufs=1) as cpool,
            tc.tile_pool(name="wp", bufs=2) as wpool,
            tc.tile_pool(name="pe", bufs=2) as pepool,
            tc.tile_pool(name="dec", bufs=2) as decpool,
            tc.tile_pool(name="abuf", bufs=2) as apool,
            tc.tile_pool(name="bbuf", bufs=2) as bpool,
            tc.tile_pool(name="ps", bufs=1, space="PSUM") as ps,
        ):
            cond_sb = cpool.tile([128, 6], BF16)
            nc.sync.dma_start(cond_sb[:], cond[:])
            bias2_sb = cpool.tile([2, L * CHUNK], BF16)
            nc.sync.dma_start(bias2_sb[:], bias2[:])
            wa1_sb = cpool.tile([128, 1024], BF16)
            nc.sync.dma_start(wa1_sb[:], wa1[:])
            wb1_sb = cpool.tile([128, 1024], BF16)
            nc.scalar.dma_start(wb1_sb[:], wb1[:])
            wa2_sb = cpool.tile([128, 2048], BF16)
            nc.sync.dma_start(wa2_sb[:], wa2[:])
            wb2_sb = cpool.tile([128, 2048], BF16)
            nc.scalar.dma_start(wb2_sb[:], wb2[:])
            ba1_sb = cpool.tile([128, 2], F32)
            nc.sync.dma_start(ba1_sb[:], ba1[:])
            bb1_sb = cpool.tile([128, 2], F32)
            nc.scalar.dma_start(bb1_sb[:], bb1[:])
            sca_sb = cpool.tile([128, 2 * LT], BF16)
            nc.sync.dma_start(sca_sb[:], sca[:])
            scb_sb = cpool.tile([128, 2 * LT], BF16)
            nc.scalar.dma_start(scb_sb[:], scb[:])
            sba2_sb = cpool.tile([16, L * 1024], BF16)
            nc.sync.dma_start(sba2_sb[:], sba2[:])
            sbb2_sb = cpool.tile([16, L * 1024], BF16)
            nc.scalar.dma_start(sbb2_sb[:], sbb2[:])
            ident_sb = cpool.tile([16, 16], BF16)
            nc.sync.dma_start(ident_sb[:], ident[:])

            def load_chunk(l):
                tiles = []
                for kc in range(3):
                    t_ = wpool.tile([128, CHUNK], BF16, tag=f"wp{kc}", name=f"wp{kc}")
                    nc.gpsimd.dma_start(
                        t_[:],
                        wp[kc * 128 : (kc + 1) * 128, l * CHUNK : (l + 1) * CHUNK],
                    )
                    tiles.append(t_)
                return tiles

            def pe_layer(l, wp_t):
                """pe rows for layer l -> peT [128 ped-chunk x4, 14 rows] bf16."""
                pest = pepool.tile([2, CHUNK], BF16, tag="pest", name="pest")
                for ltl in range(T):
                    p2 = ps.tile([2, PED], F32, tag=f"p2{ltl % 2}", name="p2")
                    for kc in range(3):
                        nc.tensor.matmul(
                            p2[:],
                            cond_sb[:, kc * 2 : kc * 2 + 2],
                            wp_t[kc][:, ltl * PED : (ltl + 1) * PED],
                            start=(kc == 0),
                            stop=(kc == 2),
                        )
                    nc.vector.tensor_add(
                        pest[0:2, ltl * PED : (ltl + 1) * PED],
                        p2[:],
                        bias2_sb[0:2, l * CHUNK + ltl * PED : l * CHUNK + (ltl + 1) * PED],
                    )
                # gather rows (b*7+t) onto partitions 0..13
                pe2 = pepool.tile([RPL, PED], BF16, tag="pe2", name="pe2")
                p16 = pe2[:, :].ap[0][0]
                pst = pest[:, :].ap[0][0]
                nc.gpsimd.dma_start(
                    bass.AP(pe2[:, :].tensor, 0, [[p16, RPL], [1, PED]]),
                    bass.AP(pest[:, :].tensor, 0, [[pst, 2], [PED, T], [1, PED]]),
                )
                peT = pepool.tile([128, 4 * RPL], BF16, tag="peT", name="peT")
                for mc in range(4):
                    tr = ps.tile([128, RPL], BF16, tag=f"tr{mc % 2}", name="tr")
                    nc.tensor.transpose(
                        tr[:],
                        pe2[:, mc * 128 : (mc + 1) * 128],
                        ident_sb[0:RPL, 0:RPL],
                    )
                    nc.vector.tensor_copy(peT[:, mc * RPL : (mc + 1) * RPL], tr[:])
                return peT

            def decode(l, peT, dec):
                """Decoder MLP for layer l -> o [14 rows, 1024] bf16 (scaled+bias)."""
                w1_sb, b1_sb, sc_sb, w2_sb, sb2_sb = (
                    (wa1_sb, ba1_sb, sca_sb, wa2_sb, sba2_sb)
                    if dec == 0
                    else (wb1_sb, bb1_sb, scb_sb, wb2_sb, sbb2_sb)
                )
                h_sb = []
                for mc in range(2):
                    hp = ps.tile([128, RPL], F32, tag=f"h{mc}", name="hp")
                    for kc in range(4):
                        nc.tensor.matmul(
                            hp[:],
                            w1_sb[:, kc * 256 + mc * 128 : kc * 256 + (mc + 1) * 128],
                            peT[:, kc * RPL : (kc + 1) * RPL],
                            start=(kc == 0),
                            stop=(kc == 3),
                        )
                    hs = decpool.tile(
                        [128, RPL], BF16, tag=f"hs{dec}{mc}", name="hs"
                    )
                    act = (
                        mybir.ActivationFunctionType.Identity
                        if SIM_GELU_IDENTITY
                        else mybir.ActivationFunctionType.Gelu
                    )
                    nc.scalar.activation(
                        hs[:], hp[:], act, bias=b1_sb[:, mc : mc + 1]
                    )
                    nc.vector.tensor_mul(
                        hs[:], hs[:], sc_sb[:, l * RPL : (l + 1) * RPL]
                    )
                    h_sb.append(hs)
                o_sb = decpool.tile([RPL, 1024], BF16, tag=f"o{dec}", name="o")
                for nh in range(2):
                    op = ps.tile([RPL, 512], F32, tag=f"o{nh}", name="op")
                    for kc in range(2):
                        nc.tensor.matmul(
                            op[:],
                            h_sb[kc][:],
                            w2_sb[:, kc * 1024 + nh * 512 : kc * 1024 + (nh + 1) * 512],
                            start=(kc == 0),
                            stop=(kc == 1),
                        )
                    nc.vector.tensor_add(
                        o_sb[:, nh * 512 : (nh + 1) * 512],
                        op[:],
                        sb2_sb[0:RPL, l * 1024 + nh * 512 : l * 1024 + (nh + 1) * 512],
                    )
                return o_sb

            def emit_layer(l, peT):
                oa = decode(l, peT, 0)
                ob = decode(l, peT, 1)

                atile = apool.tile([128, WV], BF16, tag="atile", name="atile")
                btile = bpool.tile([112, WV], BF16, tag="btile", name="btile")
                pa = oa[:, :].ap[0][0]
                pb = ob[:, :].ap[0][0]
                pax = atile[:, :].ap[0][0]
                pbx = btile[:, :].ap[0][0]
                oa_t = oa[:, :].tensor
                ob_t = ob[:, :].tensor
                atile_t = atile[:, :].tensor
                btile_t = btile[:, :].tensor

                # --- seed scatters (SBUF->SBUF) ---
                # rows are b-major stage order: row = b*7 + RT[t]
                # A t<6 (rows 1..6): partition 56b+8t+rp gets rank 2rp at col 0,
                # rank 2rp+1 at col 4096
                for b in range(2):
                    for half in range(2):
                        nc.gpsimd.dma_start(
                            bass.AP(
                                atile_t,
                                48 * b * pax + half * 4096,
                                [[pax, 48], [1, 64]],
                            ),
                            bass.AP(
                                oa_t,
                                (7 * b + 1) * pa + half * 64,
                                [[pa, 6], [128, 8], [1, 64]],
                            ),
                        )
                # A t=6 (rows 0, 7): partition 96+16b+r gets rank r at 0 and 4096
                for half in range(2):
                    nc.gpsimd.dma_start(
                        bass.AP(atile_t, 96 * pax + half * 4096, [[pax, 32], [1, 64]]),
                        bass.AP(oa_t, 0, [[7 * pa, 2], [64, 16], [1, 64]]),
                    )
                # B: partition 56b+8t+copy gets the 1024-elem base block
                nc.gpsimd.dma_start(
                    bass.AP(btile_t, 0, [[pbx, 112], [1, 1024]]),
                    bass.AP(ob_t, 0, [[pb, 14], [0, 8], [1, 1024]]),
                )

                # --- widen by doubling (A on vector, B on scalar) ---
                w = 64
                while w < 4096:
                    nc.vector.tensor_copy(atile[:, w : 2 * w], atile[:, 0:w])
                    nc.vector.tensor_copy(
                        atile[:, 4096 + w : 4096 + 2 * w], atile[:, 4096 : 4096 + w]
                    )
                    w *= 2
                w = 1024
                while w < WV:
                    nc.scalar.copy(btile[:, w : 2 * w], btile[:, 0:w])
                    w *= 2

                # --- output writes (16KB descriptors; one partition dim per AP) ---
                base = l * LAYER_SIZE
                stA45 = A_SIZES[4] + B_SIZES[4]   # A t4 -> t5 stride (241664)
                stB45 = B_SIZES[4] + A_SIZES[5]   # B t4 -> t5 stride (241664)
                for b in range(2):
                    bo = base + b * LLS
                    pA = 48 * b * pax
                    pB = 56 * b * pbx
                    # A t0
                    nc.sync.dma_start(
                        bass.AP(out, bo + OFF_A[0], [[8192, 8], [1, 8192]]),
                        bass.AP(atile_t, pA, [[pax, 8], [1, 8192]]),
                    )
                    # A t1..t3 (uniform stride 81920)
                    nc.sync.dma_start(
                        bass.AP(
                            out, bo + OFF_A[1], [[81920, 3], [8192, 8], [1, 8192]]
                        ),
                        bass.AP(atile_t, pA + 8 * pax, [[pax, 24], [1, 8192]]),
                    )
                    # A t4, t5
                    nc.sync.dma_start(
                        bass.AP(
                            out, bo + OFF_A[4], [[stA45, 2], [8192, 8], [1, 8192]]
                        ),
                        bass.AP(atile_t, pA + 32 * pax, [[pax, 16], [1, 8192]]),
                    )
                    # A t6 main: 11008-elem rows; read the periodic 8192 source
                    # as two 4096 chunks (content period 64 divides 4096)
                    nc.sync.dma_start(
                        bass.AP(
                            out, bo + OFF_A[6], [[11008, 16], [4096, 2], [1, 4096]]
                        ),
                        bass.AP(
                            atile_t,
                            (96 + 16 * b) * pax,
                            [[pax, 16], [0, 2], [1, 4096]],
                        ),
                    )
                    # A t6 tail (2816 elems per rank row)
                    nc.sync.dma_start(
                        bass.AP(
                            out, bo + OFF_A[6] + 8192, [[11008, 16], [1, 2816]]
                        ),
                        bass.AP(atile_t, (96 + 16 * b) * pax, [[pax, 16], [1, 2816]]),
                    )
                    # B piece for projection t sits at partitions 56b + 8*RT[t]
                    # B t0
                    nc.scalar.dma_start(
                        bass.AP(out, bo + OFF_B[0], [[8192, 8], [1, 8192]]),
                        bass.AP(btile_t, pB + 8 * RT[0] * pbx, [[pbx, 8], [1, 8192]]),
                    )
                    # B t1, t2: 2 reps each
                    for t in (1, 2):
                        nc.scalar.dma_start(
                            bass.AP(out, bo + OFF_B[t], [[8192, 2], [1, 8192]]),
                            bass.AP(
                                btile_t, pB + 8 * RT[t] * pbx, [[pbx, 2], [1, 8192]]
                            ),
                        )
                    # B t3
                    nc.scalar.dma_start(
                        bass.AP(out, bo + OFF_B[3], [[8192, 8], [1, 8192]]),
                        bass.AP(btile_t, pB + 8 * RT[3] * pbx, [[pbx, 8], [1, 8192]]),
                    )
                    # B t4, t5: 21.5 reps of 8192 = 8 + 8 + 5 + tail 4096.
                    # reps 0..7 read copies 0..7 of t4 then t5 (contiguous 16
                    # partitions at stage slots 5,6), twice; mid reps per t;
                    # the 4096 tail reads 2048 from copies 0 and 1 (base block
                    # period is 1024, so any 2048-prefix is correct content)
                    for rep in range(2):
                        nc.sync.dma_start(
                            bass.AP(
                                out,
                                bo + OFF_B[4] + rep * 65536,
                                [[stB45, 2], [8192, 8], [1, 8192]],
                            ),
                            bass.AP(
                                btile_t, pB + 8 * RT[4] * pbx, [[pbx, 16], [1, 8192]]
                            ),
                        )
                    for t in (4, 5):
                        nc.scalar.dma_start(
                            bass.AP(
                                out, bo + OFF_B[t] + 131072, [[8192, 5], [1, 8192]]
                            ),
                            bass.AP(
                                btile_t, pB + 8 * RT[t] * pbx, [[pbx, 5], [1, 8192]]
                            ),
                        )
                        nc.sync.dma_start(
                            bass.AP(
                                out, bo + OFF_B[t] + 172032, [[2048, 2], [1, 2048]]
                            ),
                            bass.AP(
                                btile_t, pB + 8 * RT[t] * pbx, [[pbx, 2], [1, 2048]]
                            ),
                        )
                    # B t6 (stage slot 0)
                    nc.scalar.dma_start(
                        bass.AP(out, bo + OFF_B[6], [[8192, 8], [1, 8192]]),
                        bass.AP(btile_t, pB, [[pbx, 8], [1, 8192]]),
                    )

            # ---- main pipeline ----
            chunks = [load_chunk(0), load_chunk(1)]
            for l in range(L):
                peT = pe_layer(l, chunks[l])
                if l + 2 < L:
                    chunks.append(load_chunk(l + 2))
                emit_layer(l, peT)

    nc.finalize()
    return nc


_NC = None


def _get_nc():
    global _NC
    if _NC is None:
        _NC = _build_nc()
    return _NC


def _marshal(inputs):
    """Build the per-core input maps from full inputs."""
    condition = np.asarray(inputs["condition"], np.float32)
    W_proj = np.asarray(inputs["W_proj"], np.float32)
    b_proj = np.asarray(inputs["b_proj"], np.float32)
    WA1 = np.asarray(inputs["WA1"], np.float32)
    bA1 = np.asarray(inputs["bA1"], np.float32)
    WA2 = np.asarray(inputs["WA2"], np.float32)
    bA2 = np.asarray(inputs["bA2"], np.float32)
    WB1 = np.asarray(inputs["WB1"], np.float32)
    bB1 = np.asarray(inputs["bB1"], np.float32)
    WB2 = np.asarray(inputs["WB2"], np.float32)
    bB2 = np.asarray(inputs["bB2"], np.float32)
    scales = np.asarray(inputs["scales"], np.float32)

    cond_arr = np.zeros((128, 6), np.float32)
    for kc in range(3):
        cond_arr[:, kc * 2 : kc * 2 + 2] = condition[:, kc * 128 : (kc + 1) * 128].T
    wa1_arr = np.zeros((128, 1024), np.float32)
    wb1_arr = np.zeros((128, 1024), np.float32)
    for kc in range(4):
        wa1_arr[:, kc * 256 : (kc + 1) * 256] = WA1[kc * 128 : (kc + 1) * 128, :]
        wb1_arr[:, kc * 256 : (kc + 1) * 256] = WB1[kc * 128 : (kc + 1) * 128, :]
    wa2_arr = np.zeros((128, 2048), np.float32)
    wb2_arr = np.zeros((128, 2048), np.float32)
    for kc in range(2):
        wa2_arr[:, kc * 1024 : (kc + 1) * 1024] = WA2[kc * 128 : (kc + 1) * 128, :]
        wb2_arr[:, kc * 1024 : (kc + 1) * 1024] = WB2[kc * 128 : (kc + 1) * 128, :]
    ba1_arr = np.ascontiguousarray(bA1.reshape(2, 128).T)
    bb1_arr = np.ascontiguousarray(bB1.reshape(2, 128).T)
    ident_arr = np.eye(16, dtype=np.float32)

    in_maps = []
    for c in range(NCORES):
        lt0 = c * LT
        # reorder each layer's 7 projections into stage order s (t = TOF[s])
        wp_full = W_proj[:, lt0 * PED : (lt0 + LT) * PED].reshape(EMB, L, T, PED)
        bp_full = b_proj[lt0 * PED : (lt0 + LT) * PED].reshape(L, T, PED)
        wp_c = np.ascontiguousarray(
            wp_full[:, :, TOF, :].reshape(EMB, LT * PED).astype(NPBF16)
        )
        bias2_row = np.ascontiguousarray(bp_full[:, TOF, :].reshape(L * CHUNK))
        bias2_arr = np.broadcast_to(bias2_row[None, :], (2, L * CHUNK)).copy()
        # rows are b-major stage order: row = b*7 + s, s = RT[t]
        sca_row = np.zeros(2 * LT, np.float32)
        scb_row = np.zeros(2 * LT, np.float32)
        for l in range(L):
            for t in range(T):
                for b in range(2):
                    r = l * RPL + b * T + RT[t]
                    sca_row[r] = scales[lt0 + l * T + t, 0]
                    scb_row[r] = scales[lt0 + l * T + t, 1]
        sca_arr = np.broadcast_to(sca_row[None, :], (128, 2 * LT)).copy()
        scb_arr = np.broadcast_to(scb_row[None, :], (128, 2 * LT)).copy()
        sba2_arr = np.zeros((16, L * 1024), np.float32)
        sbb2_arr = np.zeros((16, L * 1024), np.float32)
        for l in range(L):
            for t in range(T):
                for b in range(2):
                    r = b * T + RT[t]
                    sba2_arr[r, l * 1024 : (l + 1) * 1024] = (
                        scales[lt0 + l * T + t, 0] * bA2
                    )
                    sbb2_arr[r, l * 1024 : (l + 1) * 1024] = (
                        scales[lt0 + l * T + t, 1] * bB2
                    )
        in_maps.append(
            {
                "cond": cond_arr.astype(NPBF16),
                "wp": wp_c,
                "bias2": bias2_arr.astype(NPBF16),
                "wa1": wa1_arr.astype(NPBF16),
                "wb1": wb1_arr.astype(NPBF16),
                "wa2": wa2_arr.astype(NPBF16),
                "wb2": wb2_arr.astype(NPBF16),
                "ba1": ba1_arr,
                "bb1": bb1_arr,
                "sca": sca_arr.astype(NPBF16),
                "scb": scb_arr.astype(NPBF16),
                "sba2": sba2_arr.astype(NPBF16),
                "sbb2": sbb2_arr.astype(NPBF16),
                "ident": ident_arr.astype(NPBF16),
            }
        )
    return in_maps


def _ensure_ntff_hook():
    """Register the axon NTFF profile hook if the boot didn't (module was
    missing at boot time)."""
    import types

    ah = sys.modules.get("antenv.axon_hooks")
    if ah is None:
        ah = types.ModuleType("antenv.axon_hooks")
        ah._hook = None

        def _set(h, _m=ah):
            _m._hook = h

        def _get(_m=ah):
            return _m._hook

        ah.set_axon_ntff_profile_hook = _set
        ah.get_axon_ntff_profile_hook = _get
        sys.modules["antenv.axon_hooks"] = ah
        import antenv

        antenv.axon_hooks = ah
    if ah.get_axon_ntff_profile_hook() is None:
        if "/root/.axon_site" not in sys.path:
            sys.path.insert(0, "/root/.axon_site")
        from trn_agent_boot.trn_boot import _ntff_profile_via_ctypes

        hook = _ntff_profile_via_ctypes("/opt/axon/libaxon_pjrt.so")
        if hook is not None:
            ah.set_axon_ntff_profile_hook(hook)


def _run(inputs, trace=False):
    if trace:
        _ensure_ntff_hook()
    nc = _get_nc()
    in_maps = _marshal(inputs)
    res = run_bass_kernel_spmd(nc, in_maps, list(range(NCORES)), trace=trace)
    full = np.empty((2, NUM_LAYERS, LAYER_SIZE), np.float32)
    for c in range(NCORES):
        full[:, c * L : (c + 1) * L, :] = (
            res.results[c]["out"].astype(np.float32).reshape(2, L, LAYER_SIZE)
        )
    return full.reshape(2, -1), res


def kernel(**inputs) -> np.ndarray:
    out, _ = _run(inputs, trace=False)
    return out
